# revision 52
# baseline (speedup 1.0000x reference)
"""Dual-stream fused attention kernel for 8 TRN2 NeuronCores.

Reference computation (B=2, N=2048, D=512, H=8, Dh=64):
    qkv_s = x_s @ W_qkv_s (s = 1,2)  -> per-head q_s, k_s, v_s
    dots  = SCALE * (q1 k1^T + q2 k2^T)          [b, h, n, n]
    attn  = softmax(dots)
    out_s = attn @ v_s                           [b, h, n, dh]
    out   = concat(merge(out1), merge(out2), axis=1) @ W_out + b_out

Sharding: core c handles batch b = c//4 and heads {2*(c%4), 2*(c%4)+1}
(data parallel on b, tensor parallel on h). Each core computes a partial
out-projection over its 128 inner columns; the host sums the 4 partials
per batch (the TP all-reduce) and adds b_out.

On-core dataflow (all matmuls bf16, fp32 PSUM accumulation):
  - QK projections run with full-width (M=128) stationaries covering both
    heads; the PSUM is evacuated with two 64-partition copies (ACT h0 /
    DVE h1) that regroup into QT/KT [d'=128, n] per head with the streams
    stacked on the contraction dim (d' = [s1 64 | s2 64]), so scores fuse
    both streams in a single K=128 matmul.
  - Scores are computed transposed, S^T [k, q], into [128,1024] two-bank
    PSUM tiles so exp runs as one wide ACT instruction per key block.
  - Softmax is max-free (|SCALE * dots| <~ 1.5 for this data
    distribution); the denominator is accumulated on the vector engine
    (ping-pong sum of P^T tiles over k-blocks), reduced across partitions
    with a ones matmul, and inverted with the fast Newton-Raphson
    reciprocal custom op.
  - The four attention units (qb, h) are software-pipelined: unit i's
    epilogue (last AV accumulation, denominator matmul, reciprocal,
    normalize) is emitted inside unit i+1's k-loop so the ACT exp stream
    never stalls at a unit boundary and the PE stays HAM-warm.
  - Normalization: units 0-2 evacuate unnormalized AV output to SBUF
    (DVE) and multiply by the broadcast reciprocal on the otherwise-idle
    GPSIMD engine (which cannot touch PSUM); the last unit normalizes
    straight out of PSUM on DVE in 512-column halves so the tail
    out-projection starts as early as possible.
  - The projections are mostly dissolved into the attention stream: the
    pre-phase computes only the chunk-0 QK groups and the first V quad;
    the remaining V quads and chunk-1 K^T groups fill unit 0's PE slack,
    the chunk-1 Q^T groups (needed only by q-block 1) fill unit 1's.
  - The previous q-block's out-projection matmuls + staging are
    interleaved one-per-two-k-iterations into units 2-3; the last
    q-block's run in the tail per 512-column normalize half, with filler
    matmuls holding the PE HAM-warm, PSUMs rotating through both the
    scores ring and the freed AV banks, and paired two-row-block output
    DMAs alternating across queues.
"""

import numpy as np
import ml_dtypes

import bass_rust
import concourse.bass as bass
import concourse.mybir as mybir
import concourse.tile as tile
from concourse.vector_clock import ScopedClock
from concourse.bass_utils import run_bass_kernel_spmd

B, N, D = 2, 2048, 512
H, DH = 8, 64
SCALE = (2 * DH) ** -0.5
NCORES = 8
HPC = 2              # heads per core
CW = HPC * DH        # 128: per-core slice width of the inner dim
DC = D // 128        # 4 contraction chunks for the projections
NKB = N // 128       # 16 key blocks
QB = 1024            # q-block width for the attention inner loop
NQB = N // QB        # 2
BF16 = ml_dtypes.bfloat16


_WAIT_LIMIT = 1  # this container's walrus rejects multiple sync waits per instruction


def _split_sync_waits(nc):
    """Hoist excess semaphore waits onto same-engine NOPs inserted right
    before the over-budget instruction ("Too many sync wait commands")."""
    for f in nc.m.functions:
        for bb in f.blocks:
            insts = bb.instructions
            i = 0
            while i < len(insts):
                inst = insts[i]
                si = inst.sync_info
                if si is None:
                    i += 1
                    continue
                waits = list(si.on_wait)
                sem_waits = [w for w in waits if w.sync_type == "semaphore"]
                other = [w for w in waits if w.sync_type != "semaphore"]
                budget = _WAIT_LIMIT - len(other)
                if len(sem_waits) <= budget:
                    i += 1
                    continue
                keep = sem_waits[-budget:] if budget > 0 else []
                extra = sem_waits[:-budget] if budget > 0 else sem_waits
                for j in range(0, len(extra), _WAIT_LIMIT):
                    nop = mybir.InstNoOp(
                        name=f"I-{nc.next_id()}",
                        engine=inst.engine,
                        bass_nofuse=True,
                        sync_info=mybir.SyncInfo(
                            on_wait=extra[j:j + _WAIT_LIMIT], on_update=[]
                        ),
                    )
                    insts.insert(i, nop)
                    i += 1
                si.on_wait = other + keep
                inst.sync_info = si
                i += 1


def _copy_on(eng, nc, out, in_):
    if eng is nc.scalar:
        eng.copy(out=out, in_=in_)
    else:
        eng.tensor_copy(out=out, in_=in_)


def _body(nc, tc):
    bf = mybir.dt.bfloat16
    f32 = mybir.dt.float32
    EXP = mybir.ActivationFunctionType.Exp

    x1T = nc.dram_tensor("x1T", [D, N], bf, kind="ExternalInput").ap()
    x2T = nc.dram_tensor("x2T", [D, N], bf, kind="ExternalInput").ap()
    wq = [nc.dram_tensor(f"wq{s}", [D, CW], bf, kind="ExternalInput").ap() for s in (1, 2)]
    wk = [nc.dram_tensor(f"wk{s}", [D, CW], bf, kind="ExternalInput").ap() for s in (1, 2)]
    wv = [nc.dram_tensor(f"wv{s}", [D, CW], bf, kind="ExternalInput").ap() for s in (1, 2)]
    wout = nc.dram_tensor("wout", [CW, D], bf, kind="ExternalInput").ap()
    out = nc.dram_tensor("out", [2 * N, D], bf, kind="ExternalOutput").ap()
    xT = [x1T, x2T]

    pools = []

    def mkpool(**kw):
        p = tc.alloc_tile_pool(**kw)
        pools.append(p)
        return p

    singles = mkpool(name="singles", bufs=1)
    spool = mkpool(name="spool", bufs=2, space="PSUM")      # 2x [128,1024] = 4 banks
    avpool = mkpool(name="avpool", bufs=2, space="PSUM")    # 2x [128,1024] = 4 banks
    ptpool = mkpool(name="ptpool", bufs=12)
    accpool = mkpool(name="accpool", bufs=2)
    bcpool = mkpool(name="bcpool", bufs=2)
    unpool = mkpool(name="unpool", bufs=2)
    ostage = mkpool(name="ostage", bufs=9)

    # ---- resident inputs -------------------------------------------------
    # the first projection group needs wq1/wk1 + x1 chunk 0 (all 4 d-rows);
    # order descriptors so those land first across the three DMA queues.
    def load_w(ap, name, eng):
        t = singles.tile([128, DC, CW], bf, tag=name, name=name)
        eng.dma_start(out=t, in_=ap.rearrange("(dc p) c -> p dc c", p=128))
        return t

    x_sb = [[singles.tile([128, N], bf, tag=f"x{s}_{dc}", name=f"x{s}_{dc}")
             for dc in range(DC)] for s in range(2)]

    def load_x(s, dc, chunk, eng, half=None):
        c0 = chunk * 1024 if half is None else chunk * 1024 + half * 512
        w = 1024 if half is None else 512
        eng.dma_start(out=x_sb[s][dc][:, c0:c0 + w],
                      in_=xT[s][dc * 128:(dc + 1) * 128, c0:c0 + w])

    # warm the PE HAM clock gate during the DMA-bound start: the first real
    # matmuls land ~13us in and would otherwise run their first ~3.4us at
    # K=4/8. These junk matmuls need no input data and their PSUM region is
    # overwritten (start=True) by the first projection group.
    ones_mat = singles.tile([128, 128], bf, tag="ones", name="ones")
    nc.vector.memset(ones_mat, 1.0)
    warm_ps = spool.tile([128, 1024], f32, tag="s", name="warm")
    for _ in range(24):
        nc.tensor.matmul(warm_ps[:, 0:128], lhsT=ones_mat, rhs=ones_mat,
                         start=True, stop=True)

    # interleave descriptors across the three DMA-capable engines so the
    # first projection group's operands (wq1/wk1 + x1 chunk 0) land first
    wq_sb = [None, None]
    wk_sb = [None, None]
    wv_sb = [None, None]
    # the first projection group needs only wq1 + x1 columns 0:512; issue
    # those as small half-chunk transfers so the PE starts earliest
    wq_sb[0] = load_w(wq[0], "wq1", nc.sync)       # sync #1
    load_x(0, 1, 0, nc.gpsimd, half=0)             # gp   #1
    load_x(0, 2, 0, nc.scalar, half=0)             # sc   #1
    load_x(0, 0, 0, nc.sync, half=0)               # sync #2
    load_x(0, 3, 0, nc.gpsimd, half=0)             # gp   #2
    load_x(0, 2, 0, nc.scalar, half=1)             # sc   #2
    load_x(0, 0, 0, nc.sync, half=1)               # sync #3
    load_x(0, 1, 0, nc.gpsimd, half=1)             # gp   #3
    load_x(0, 3, 0, nc.scalar, half=1)             # sc   #3
    wk_sb[0] = load_w(wk[0], "wk1", nc.sync)       # sync #4
    wq_sb[1] = load_w(wq[1], "wq2", nc.gpsimd)     # gp   #4
    wk_sb[1] = load_w(wk[1], "wk2", nc.scalar)     # sc   #4
    load_x(1, 0, 0, nc.sync)                       # sync #5
    load_x(1, 1, 0, nc.gpsimd)                     # gp   #5
    load_x(1, 2, 0, nc.scalar)                     # sc   #5
    load_x(1, 3, 0, nc.gpsimd)                     # gp   #6
    wv_sb[0] = load_w(wv[0], "wv1", nc.gpsimd)     # gp   #5
    wv_sb[1] = load_w(wv[1], "wv2", nc.scalar)     # sc   #5
    load_x(0, 0, 1, nc.sync)
    load_x(0, 1, 1, nc.gpsimd)
    load_x(0, 2, 1, nc.scalar)
    load_x(0, 3, 1, nc.sync)
    load_x(1, 0, 1, nc.gpsimd)
    load_x(1, 1, 1, nc.scalar)
    load_x(1, 2, 1, nc.sync)
    load_x(1, 3, 1, nc.gpsimd)
    wout_sb = singles.tile([CW, D], bf, tag="wout", name="wout")
    nc.gpsimd.dma_start(out=wout_sb, in_=wout)


    # ---- QK projections: QT/KT [128 = (s1 dh | s2 dh), N] per head -------
    qt = [singles.tile([128, N], bf, tag=f"qt{h}", name=f"qt{h}") for h in range(HPC)]
    kt = [singles.tile([128, N], bf, tag=f"kt{h}", name=f"kt{h}") for h in range(HPC)]
    v_all = singles.tile([128, NKB, HPC, 2, DH], bf, tag="vall", name="vall")

    def qk_group(chunk, s, dst, w_sb):
        ps = spool.tile([128, 1024], f32, tag="s", name="ps")
        for half in range(2):
            c0 = chunk * 1024 + half * 512
            for dc in range(DC):
                nc.tensor.matmul(
                    ps[:, half * 512:(half + 1) * 512],
                    lhsT=w_sb[s][:, dc, :],
                    rhs=x_sb[s][dc][:, c0:c0 + 512],
                    start=(dc == 0),
                    stop=(dc == DC - 1),
                )
        for h in range(HPC):
            eng = nc.scalar if h == 0 else nc.vector
            _copy_on(eng, nc,
                     out=dst[h][s * 64:(s + 1) * 64,
                                chunk * 1024:(chunk + 1) * 1024],
                     in_=ps[h * 64:(h + 1) * 64, :])

    # V projection in quads: four 128-row blocks per PSUM tile, one wide
    # evacuation copy (alternating ACT/DVE)
    vq_state = {}

    def vproj_pair(quad, pos, eng_ix):
        if pos == 0:
            vq_state[quad] = spool.tile([128, 1024], f32, tag="s", name="vps")
        ps = vq_state[quad]
        for i in range(2):
            nb = quad * 4 + pos * 2 + i
            for s in range(2):
                for dc in range(DC):
                    nc.tensor.matmul(
                        ps[:, (pos * 2 + i) * 256 + s * CW:
                           (pos * 2 + i) * 256 + (s + 1) * CW],
                        lhsT=x_sb[s][dc][:, nb * 128:(nb + 1) * 128],
                        rhs=wv_sb[s][:, dc, :],
                        start=(dc == 0),
                        stop=(dc == DC - 1),
                    )
        if pos == 1:
            # ISA free-dim patterns are limited to 3D: one copy per head
            for h in range(HPC):
                eng = nc.vector if (eng_ix + h) % 2 == 0 else nc.scalar
                _copy_on(eng, nc,
                         out=v_all[:, quad * 4:(quad + 1) * 4, h, :, :],
                         in_=ps.rearrange("p (n s h d) -> p n h s d",
                                          n=4, s=2, h=HPC)[:, :, h, :, :])

    # pre-phase: only the chunk-0 QK projections — the minimum for unit 0's
    # first score matmuls. Everything else (all V quads, chunk-1 QK groups)
    # is interleaved into units 0-1 so the PE never idles into a HAM
    # re-throttle between the projections and the attention stream.
    for s in range(2):
        for dst, w_sb in ((qt, wq_sb), (kt, wk_sb)):
            qk_group(0, s, dst, w_sb)

    # ---- attention -------------------------------------------------------
    # merged[s]: [128 = (h0 dh | h1 dh), N] per stream, normalized.
    merged = [singles.tile([128, N], bf, tag=f"merged{s}", name=f"merged{s}") for s in range(2)]

    o_stage = [None]

    def outproj_step(qb_src, idx, stage_engines, dma_eng, pair_pos, pool=None,
                     keep_warm=False):
        # idx 0..15 -> stream s = idx//8, row block rb within the q-block;
        # pair_pos 0/1: two consecutive same-stream row blocks share one DMA
        s, r = divmod(idx, 8)
        rb = qb_src * (QB // 128) + r
        if pool is None:
            ps = spool.tile([128, 1024], f32, tag="s", name="ps")
        else:
            ps = pool.tile([128, 1024], f32, tag="av", name="ps")
        nc.tensor.matmul(
            ps[:, 0:512],
            lhsT=merged[s][:, rb * 128:(rb + 1) * 128],
            rhs=wout_sb,
            start=True,
            stop=True,
        )
        if keep_warm:
            # the tail out-projection is staging-gated (~50% PE duty) which
            # lets HAM re-throttle the PE clock; burn a junk matmul into the
            # unused PSUM half to hold the busy window
            nc.tensor.matmul(
                ps[:, 512:1024],
                lhsT=ones_mat,
                rhs=qt[0][:, 0:512],
                start=True,
                stop=True,
            )
        # DMA cannot read PSUM; stage via SBUF, two row-blocks per DMA.
        if pair_pos == 0:
            o_stage[0] = ostage.tile([128, 2, 512], bf, tag="ost", name="ost")
        _copy_on(stage_engines[pair_pos], nc,
                 out=o_stage[0][:, pair_pos, :], in_=ps[:, 0:512])
        if pair_pos == 1:
            r0 = s * N + (rb - 1) * 128
            dma_eng.dma_start(
                out=out[r0:r0 + 256, :].rearrange("(b p) c -> p b c", p=128),
                in_=o_stage[0],
            )

    units = [(qb, h) for qb in range(NQB) for h in range(HPC)]
    pending = [None]
    avtail = [None]

    def make_epilogue(qb, h, av, acc):
        q0 = qb * QB

        def epilogue():
            bc = spool.tile([128, 1024], f32, tag="s", name="bc")
            for qh in range(2):
                nc.tensor.matmul(bc[:, qh * 512:(qh + 1) * 512], lhsT=ones_mat,
                                 rhs=acc[1][:, qh * 512:(qh + 1) * 512],
                                 start=True, stop=True)
            # unnormalized evac frees the AV PSUM; normalize on the
            # otherwise-idle gpsimd engine (it cannot read PSUM)
            un = unpool.tile([128, 1024], bf, tag="un", name="un")
            nc.vector.tensor_copy(out=un, in_=av)
            rec32 = bcpool.tile([128, 1024], f32, tag="rec32", name="rec32")
            nc.vector.reciprocal_approx_fast(out=rec32, in_=bc)
            bc16 = bcpool.tile([128, 1024], bf, tag="bc16", name="bc16")
            nc.vector.tensor_copy(out=bc16, in_=rec32)
            # split per 512-half so the first merged columns land early for
            # the interleaved out-projection of the previous q-block
            for qh in range(2):
                sl = slice(qh * 512, (qh + 1) * 512)
                for s in range(2):
                    nc.gpsimd.tensor_mul(
                        out=merged[s][h * 64:(h + 1) * 64,
                                      q0 + qh * 512:q0 + (qh + 1) * 512],
                        in0=un[s * 64:(s + 1) * 64, sl],
                        in1=bc16[s * 64:(s + 1) * 64, sl],
                    )

        return epilogue

    def emit_av(av, h, pts, kb):
        for qh in range(2):
            nc.tensor.matmul(
                av[:, qh * 512:(qh + 1) * 512],
                lhsT=v_all[:, kb, h, :, :],
                rhs=pts[kb][:, qh * 512:(qh + 1) * 512],
                start=(kb == 0),
                stop=(kb == NKB - 1),
            )

    def emit_add(acc, pts, kb):
        # denominator accumulation (ping-pong, non-in-place) on DVE
        if kb == 1:
            acc[1] = accpool.tile([128, 1024], bf, tag="acc1", name="acc1")
            nc.vector.tensor_add(out=acc[1], in0=pts[0], in1=pts[1])
        elif kb >= 2:
            if acc[kb % 2] is None:
                acc[kb % 2] = accpool.tile([128, 1024], bf,
                                           tag=f"acc{kb % 2}", name="acc")
            nc.vector.tensor_add(out=acc[kb % 2], in0=acc[1 - kb % 2],
                                 in1=pts[kb])

    for u_idx, (qb, h) in enumerate(units):
        q0 = qb * QB
        av = avpool.tile([128, 1024], f32, tag="av", name="av")
        acc = [None, None]  # ping-pong accumulators
        # previous q-block's out-projection interleave order: the rows that
        # only need the qh0 half of unit1's normalize go to unit2, the qh1
        # rows to unit3 (so no PE wait on the gpsimd normalize)
        # previous q-block's out-projection: spread one matmul per two
        # k-iterations across units 2-3 so the PE never co-paces the exp
        # stream; unit2 gets only rows whose normalize half lands early
        if u_idx == 2:
            op_sched = {kb: i for i, kb in enumerate([5, 7, 9, 11, 13, 15])}
            op_order = [0, 1, 8, 9, 2, 3]
        else:
            op_sched = {kb: i for i, kb in
                        enumerate([1, 2, 3, 4, 5, 7, 9, 11, 13, 15])}
            op_order = [10, 11, 4, 5, 12, 13, 6, 7, 14, 15]
        pts = {}
        for kb in range(NKB):
            s_ps = spool.tile([128, 1024], f32, tag="s", name="s")
            # the exp stream is paced by these; keep them at the head of
            # the PE queue
            with tc.high_priority(offset=1 << 20):
                for qh in range(2):
                    nc.tensor.matmul(
                        s_ps[:, qh * 512:(qh + 1) * 512],
                        lhsT=kt[h][:, kb * 128:(kb + 1) * 128],
                        rhs=qt[h][:, q0 + qh * 512:q0 + (qh + 1) * 512],
                        start=True,
                        stop=True,
                    )
            pt = ptpool.tile([128, 1024], bf, tag="pt", name="pt")
            nc.scalar.activation(out=pt, in_=s_ps, func=EXP, scale=SCALE)
            pts[kb] = pt
            # previous unit's AV/add tail goes here so it never blocks this
            # unit's score matmuls in the PE queue
            if kb == 0 and avtail[0] is not None:
                avtail[0]()
                avtail[0] = None
            if kb == 2 and pending[0] is not None:
                pending[0]()
                pending[0] = None
            if u_idx == 0:
                # fill unit 0's PE slack with the V quads and the chunk-1
                # KT projection groups (needed from kb 8 on); quad0 goes
                # first, right after the first scores/exp, so the exp
                # stream opens ~5us earlier than a serial pre-phase
                if kb == 0:
                    vproj_pair(0, 0, 0)
                    vproj_pair(0, 1, 0)
                elif kb in (1, 2):
                    vproj_pair(1, kb - 1, kb)
                elif kb == 3:
                    qk_group(1, 0, kt, wk_sb)
                elif kb in (4, 5):
                    vproj_pair(2, kb - 4, kb)
                elif kb == 6:
                    qk_group(1, 1, kt, wk_sb)
                elif kb in (7, 8):
                    vproj_pair(3, kb - 7, kb)
            elif u_idx == 1:
                # chunk-1 QT groups are only needed by unit 2
                if kb == 1:
                    qk_group(1, 0, qt, wq_sb)
                elif kb == 3:
                    qk_group(1, 1, qt, wq_sb)
            if qb == 1 and kb in op_sched:
                # previous q-block's out-projection (after this iteration's
                # scores so a normalize wait can never stall the exp stream;
                # stage copies on DVE, DMAs on sync)
                i = op_sched[kb]
                outproj_step(0, op_order[i], (nc.vector, nc.vector),
                             nc.sync, i % 2)
            if kb >= 1:
                emit_av(av, h, pts, kb - 1)
                emit_add(acc, pts, kb - 1)
        avtail[0] = (lambda av=av, h=h, acc=acc, pts=pts:
                     (emit_av(av, h, pts, NKB - 1), emit_add(acc, pts, NKB - 1)))
        if u_idx < len(units) - 1:
            pending[0] = make_epilogue(qb, h, av, acc)
        else:
            last_av, last_acc = av, acc

    # ---- tail: last unit's epilogue fused with its out-projection --------
    avtail[0]()
    avtail[0] = None
    qb, h = units[-1]
    q0 = qb * QB
    bc = spool.tile([128, 1024], f32, tag="s", name="bc")
    bcast = bcpool.tile([128, 1024], f32, tag="rec32", name="bcast")
    for qh in range(2):
        nc.tensor.matmul(bc[:, qh * 512:(qh + 1) * 512], lhsT=ones_mat,
                         rhs=last_acc[1][:, qh * 512:(qh + 1) * 512],
                         start=True, stop=True)
    tail_order = [(0, 1), (8, 9), (2, 3), (10, 11), (4, 5), (12, 13), (6, 7), (14, 15)]
    tail_dma = [nc.sync, nc.scalar]
    for qh in range(2):
        sl = slice(qh * 512, (qh + 1) * 512)
        nc.vector.reciprocal_approx_fast(out=bcast[:, sl], in_=bc[:, sl])
        for s in range(2):
            nc.vector.tensor_mul(
                out=merged[s][h * 64:(h + 1) * 64,
                              q0 + qh * 512:q0 + (qh + 1) * 512],
                in0=last_av[s * 64:(s + 1) * 64, sl],
                in1=bcast[s * 64:(s + 1) * 64, sl],
            )
        # row blocks covered by this half can project immediately; psums
        # alternate between the scores ring and the now-free AV banks so
        # the matmuls are not gated on the staging copies
        for n_pair, (i0, i1) in enumerate(tail_order[qh * 4:(qh + 1) * 4]):
            # AV banks are only safe to reuse once every read of the last
            # AV tile has been emitted (i.e. in the qh==1 wave)
            pool = avpool if (qh == 1 and n_pair % 2 == 1) else None
            outproj_step(NQB - 1, i0, (nc.scalar, nc.vector),
                         tail_dma[n_pair % 2], 0, pool=pool, keep_warm=True)
            outproj_step(NQB - 1, i1, (nc.scalar, nc.vector),
                         tail_dma[n_pair % 2], 1, pool=pool, keep_warm=True)

    for p in reversed(pools):
        p.release()


_NC_CACHE = None


def _build():
    global _NC_CACHE
    if _NC_CACHE is None:
        nc = bass.Bass("TRN2", target_bir_lowering=False, debug=False)
        with tile.TileContext(nc) as tc:
            _body(nc, tc)
        # populate .instr bytes for extended InstISA subclasses (the custom
        # DVE reciprocal) — raw bass skips this pass
        mybir.codegen_inst_isa_subclasses(nc)
        _split_sync_waits(nc)
        _NC_CACHE = nc
    return _NC_CACHE


def _prep_in_maps(x1, x2, W_qkv1, W_qkv2, W_out):
    x1 = np.asarray(x1, np.float32)
    x2 = np.asarray(x2, np.float32)
    W1 = np.asarray(W_qkv1, np.float32).astype(BF16)
    W2 = np.asarray(W_qkv2, np.float32).astype(BF16)
    Wo = np.asarray(W_out, np.float32).astype(BF16)
    xT = [
        [np.ascontiguousarray(x[b].T).astype(BF16) for b in range(B)]
        for x in (x1, x2)
    ]
    in_maps = []
    for c in range(NCORES):
        b, hg = divmod(c, NCORES // B)
        cs = slice(hg * CW, (hg + 1) * CW)
        in_maps.append({
            "x1T": xT[0][b],
            "x2T": xT[1][b],
            "wq1": np.ascontiguousarray(W1[:, 0:D][:, cs]),
            "wq2": np.ascontiguousarray(W2[:, 0:D][:, cs]),
            "wk1": np.ascontiguousarray(W1[:, D:2 * D][:, cs]),
            "wk2": np.ascontiguousarray(W2[:, D:2 * D][:, cs]),
            "wv1": np.ascontiguousarray(W1[:, 2 * D:3 * D][:, cs]),
            "wv2": np.ascontiguousarray(W2[:, 2 * D:3 * D][:, cs]),
            "wout": np.ascontiguousarray(Wo[cs, :]),
        })
    return in_maps


def _run(inputs, **spmd_kwargs):
    nc = _build()
    in_maps = _prep_in_maps(
        inputs["x1"], inputs["x2"], inputs["W_qkv1"], inputs["W_qkv2"],
        inputs["W_out"],
    )
    res = run_bass_kernel_spmd(nc, in_maps, core_ids=list(range(NCORES)),
                               **spmd_kwargs)
    b_out = np.asarray(inputs["b_out"], np.float32)
    gpc = NCORES // B
    full = np.zeros((B, 2 * N, D), np.float32)
    for c in range(NCORES):
        full[c // gpc] += res.results[c]["out"].astype(np.float32)
    full += b_out
    return full, res


def kernel(**inputs):
    full, _ = _run(inputs)
    return full


# revision 54
# speedup vs baseline: 1.0080x; 1.0080x over previous
"""Dual-stream fused attention kernel for 8 TRN2 NeuronCores.

Reference computation (B=2, N=2048, D=512, H=8, Dh=64):
    qkv_s = x_s @ W_qkv_s (s = 1,2)  -> per-head q_s, k_s, v_s
    dots  = SCALE * (q1 k1^T + q2 k2^T)          [b, h, n, n]
    attn  = softmax(dots)
    out_s = attn @ v_s                           [b, h, n, dh]
    out   = concat(merge(out1), merge(out2), axis=1) @ W_out + b_out

Sharding: core c handles batch b = c//4 and heads {2*(c%4), 2*(c%4)+1}
(data parallel on b, tensor parallel on h). Each core computes a partial
out-projection over its 128 inner columns; the host sums the 4 partials
per batch (the TP all-reduce) and adds b_out.

On-core dataflow (all matmuls bf16, fp32 PSUM accumulation):
  - QK projections run with full-width (M=128) stationaries covering both
    heads; the PSUM is evacuated with two 64-partition copies (ACT h0 /
    DVE h1) that regroup into QT/KT [d'=128, n] per head with the streams
    stacked on the contraction dim (d' = [s1 64 | s2 64]), so scores fuse
    both streams in a single K=128 matmul.
  - Scores are computed transposed, S^T [k, q], into [128,1024] two-bank
    PSUM tiles so exp runs as one wide ACT instruction per key block.
  - Softmax is max-free (|SCALE * dots| <~ 1.5 for this data
    distribution); the denominator is accumulated on the vector engine
    (ping-pong sum of P^T tiles over k-blocks), reduced across partitions
    with a ones matmul, and inverted with the fast Newton-Raphson
    reciprocal custom op.
  - The four attention units (qb, h) are software-pipelined: unit i's
    epilogue (last AV accumulation, denominator matmul, reciprocal,
    normalize) is emitted inside unit i+1's k-loop so the ACT exp stream
    never stalls at a unit boundary and the PE stays HAM-warm.
  - Normalization: units 0-2 evacuate unnormalized AV output to SBUF
    (DVE) and multiply by the broadcast reciprocal on the otherwise-idle
    GPSIMD engine (which cannot touch PSUM); the last unit normalizes
    straight out of PSUM on DVE in 512-column halves so the tail
    out-projection starts as early as possible.
  - The projections are mostly dissolved into the attention stream: the
    pre-phase computes only the chunk-0 QK groups and the first V quad;
    the remaining V quads and chunk-1 K^T groups fill unit 0's PE slack,
    the chunk-1 Q^T groups (needed only by q-block 1) fill unit 1's.
  - The previous q-block's out-projection matmuls + staging are
    interleaved one-per-two-k-iterations into units 2-3; the last
    q-block's run in the tail per 512-column normalize half, with filler
    matmuls holding the PE HAM-warm, PSUMs rotating through both the
    scores ring and the freed AV banks, and paired two-row-block output
    DMAs alternating across queues.
"""

import numpy as np
import ml_dtypes

import bass_rust
import concourse.bass as bass
import concourse.mybir as mybir
import concourse.tile as tile
from concourse.vector_clock import ScopedClock
from concourse.bass_utils import run_bass_kernel_spmd

B, N, D = 2, 2048, 512
H, DH = 8, 64
SCALE = (2 * DH) ** -0.5
NCORES = 8
HPC = 2              # heads per core
CW = HPC * DH        # 128: per-core slice width of the inner dim
DC = D // 128        # 4 contraction chunks for the projections
NKB = N // 128       # 16 key blocks
QB = 1024            # q-block width for the attention inner loop
NQB = N // QB        # 2
BF16 = ml_dtypes.bfloat16


_WAIT_LIMIT = 1  # this container's walrus rejects multiple sync waits per instruction


def _split_sync_waits(nc):
    """Hoist excess semaphore waits onto same-engine NOPs inserted right
    before the over-budget instruction ("Too many sync wait commands")."""
    for f in nc.m.functions:
        for bb in f.blocks:
            insts = bb.instructions
            i = 0
            while i < len(insts):
                inst = insts[i]
                si = inst.sync_info
                if si is None:
                    i += 1
                    continue
                waits = list(si.on_wait)
                sem_waits = [w for w in waits if w.sync_type == "semaphore"]
                other = [w for w in waits if w.sync_type != "semaphore"]
                budget = _WAIT_LIMIT - len(other)
                if len(sem_waits) <= budget:
                    i += 1
                    continue
                keep = sem_waits[-budget:] if budget > 0 else []
                extra = sem_waits[:-budget] if budget > 0 else sem_waits
                for j in range(0, len(extra), _WAIT_LIMIT):
                    nop = mybir.InstNoOp(
                        name=f"I-{nc.next_id()}",
                        engine=inst.engine,
                        bass_nofuse=True,
                        sync_info=mybir.SyncInfo(
                            on_wait=extra[j:j + _WAIT_LIMIT], on_update=[]
                        ),
                    )
                    insts.insert(i, nop)
                    i += 1
                si.on_wait = other + keep
                inst.sync_info = si
                i += 1


def _copy_on(eng, nc, out, in_):
    if eng is nc.scalar:
        eng.copy(out=out, in_=in_)
    else:
        eng.tensor_copy(out=out, in_=in_)


def _body(nc, tc):
    bf = mybir.dt.bfloat16
    f32 = mybir.dt.float32
    EXP = mybir.ActivationFunctionType.Exp

    x1T = nc.dram_tensor("x1T", [D, N], bf, kind="ExternalInput").ap()
    x2T = nc.dram_tensor("x2T", [D, N], bf, kind="ExternalInput").ap()
    wq = [nc.dram_tensor(f"wq{s}", [D, CW], bf, kind="ExternalInput").ap() for s in (1, 2)]
    wk = [nc.dram_tensor(f"wk{s}", [D, CW], bf, kind="ExternalInput").ap() for s in (1, 2)]
    wv = [nc.dram_tensor(f"wv{s}", [D, CW], bf, kind="ExternalInput").ap() for s in (1, 2)]
    wout = nc.dram_tensor("wout", [CW, D], bf, kind="ExternalInput").ap()
    out = nc.dram_tensor("out", [2 * N, D], bf, kind="ExternalOutput").ap()
    xT = [x1T, x2T]

    pools = []

    def mkpool(**kw):
        p = tc.alloc_tile_pool(**kw)
        pools.append(p)
        return p

    singles = mkpool(name="singles", bufs=1)
    spool = mkpool(name="spool", bufs=2, space="PSUM")      # 2x [128,1024] = 4 banks
    avpool = mkpool(name="avpool", bufs=2, space="PSUM")    # 2x [128,1024] = 4 banks
    ptpool = mkpool(name="ptpool", bufs=12)
    accpool = mkpool(name="accpool", bufs=2)
    bcpool = mkpool(name="bcpool", bufs=2)
    unpool = mkpool(name="unpool", bufs=2)
    ostage = mkpool(name="ostage", bufs=9)

    # ---- resident inputs -------------------------------------------------
    # the first projection group needs wq1/wk1 + x1 chunk 0 (all 4 d-rows);
    # order descriptors so those land first across the three DMA queues.
    def load_w(ap, name, eng):
        t = singles.tile([128, DC, CW], bf, tag=name, name=name)
        eng.dma_start(out=t, in_=ap.rearrange("(dc p) c -> p dc c", p=128))
        return t

    x_sb = [[singles.tile([128, N], bf, tag=f"x{s}_{dc}", name=f"x{s}_{dc}")
             for dc in range(DC)] for s in range(2)]

    def load_x(s, dc, chunk, eng, half=None):
        c0 = chunk * 1024 if half is None else chunk * 1024 + half * 512
        w = 1024 if half is None else 512
        eng.dma_start(out=x_sb[s][dc][:, c0:c0 + w],
                      in_=xT[s][dc * 128:(dc + 1) * 128, c0:c0 + w])

    # warm the PE HAM clock gate during the DMA-bound start: the first real
    # matmuls land ~13us in and would otherwise run their first ~3.4us at
    # K=4/8. These junk matmuls need no input data and their PSUM region is
    # overwritten (start=True) by the first projection group.
    ones_mat = singles.tile([128, 128], bf, tag="ones", name="ones")
    nc.vector.memset(ones_mat, 1.0)
    warm_ps = spool.tile([128, 1024], f32, tag="s", name="warm")
    for _ in range(24):
        nc.tensor.matmul(warm_ps[:, 0:128], lhsT=ones_mat, rhs=ones_mat,
                         start=True, stop=True)

    # interleave descriptors across the three DMA-capable engines so the
    # first projection group's operands (wq1/wk1 + x1 chunk 0) land first
    wq_sb = [None, None]
    wk_sb = [None, None]
    wv_sb = [None, None]
    # the first projection group needs only wq1 + x1 columns 0:512; issue
    # those as small half-chunk transfers so the PE starts earliest
    wq_sb[0] = load_w(wq[0], "wq1", nc.sync)       # sync #1
    load_x(0, 1, 0, nc.gpsimd, half=0)             # gp   #1
    load_x(0, 2, 0, nc.scalar, half=0)             # sc   #1
    load_x(0, 0, 0, nc.sync, half=0)               # sync #2
    load_x(0, 3, 0, nc.gpsimd, half=0)             # gp   #2
    load_x(0, 2, 0, nc.scalar, half=1)             # sc   #2
    load_x(0, 0, 0, nc.sync, half=1)               # sync #3
    load_x(0, 1, 0, nc.gpsimd, half=1)             # gp   #3
    load_x(0, 3, 0, nc.scalar, half=1)             # sc   #3
    wk_sb[0] = load_w(wk[0], "wk1", nc.sync)       # sync #4
    wq_sb[1] = load_w(wq[1], "wq2", nc.gpsimd)     # gp   #4
    wk_sb[1] = load_w(wk[1], "wk2", nc.scalar)     # sc   #4
    load_x(1, 0, 0, nc.sync)                       # sync #5
    load_x(1, 1, 0, nc.gpsimd)                     # gp   #5
    load_x(1, 2, 0, nc.scalar)                     # sc   #5
    load_x(1, 3, 0, nc.gpsimd)                     # gp   #6
    wv_sb[0] = load_w(wv[0], "wv1", nc.gpsimd)     # gp   #5
    wv_sb[1] = load_w(wv[1], "wv2", nc.scalar)     # sc   #5
    load_x(0, 0, 1, nc.sync)
    load_x(0, 1, 1, nc.gpsimd)
    load_x(0, 2, 1, nc.scalar)
    load_x(0, 3, 1, nc.sync)
    load_x(1, 0, 1, nc.gpsimd)
    load_x(1, 1, 1, nc.scalar)
    load_x(1, 2, 1, nc.sync)
    load_x(1, 3, 1, nc.gpsimd)
    wout_sb = singles.tile([CW, D], bf, tag="wout", name="wout")
    nc.gpsimd.dma_start(out=wout_sb, in_=wout)


    # ---- QK projections: QT/KT [128 = (s1 dh | s2 dh), N] per head -------
    qt = [singles.tile([128, N], bf, tag=f"qt{h}", name=f"qt{h}") for h in range(HPC)]
    kt = [singles.tile([128, N], bf, tag=f"kt{h}", name=f"kt{h}") for h in range(HPC)]
    v_all = singles.tile([128, NKB, HPC, 2, DH], bf, tag="vall", name="vall")

    def qk_group(chunk, s, dst, w_sb):
        ps = spool.tile([128, 1024], f32, tag="s", name="ps")
        for half in range(2):
            c0 = chunk * 1024 + half * 512
            for dc in range(DC):
                nc.tensor.matmul(
                    ps[:, half * 512:(half + 1) * 512],
                    lhsT=w_sb[s][:, dc, :],
                    rhs=x_sb[s][dc][:, c0:c0 + 512],
                    start=(dc == 0),
                    stop=(dc == DC - 1),
                )
        for h in range(HPC):
            eng = nc.scalar if h == 0 else nc.vector
            _copy_on(eng, nc,
                     out=dst[h][s * 64:(s + 1) * 64,
                                chunk * 1024:(chunk + 1) * 1024],
                     in_=ps[h * 64:(h + 1) * 64, :])

    # V projection in quads: four 128-row blocks per PSUM tile, one wide
    # evacuation copy (alternating ACT/DVE)
    vq_state = {}

    def vproj_pair(quad, pos, eng_ix):
        if pos == 0:
            vq_state[quad] = spool.tile([128, 1024], f32, tag="s", name="vps")
        ps = vq_state[quad]
        for i in range(2):
            nb = quad * 4 + pos * 2 + i
            for s in range(2):
                for dc in range(DC):
                    nc.tensor.matmul(
                        ps[:, (pos * 2 + i) * 256 + s * CW:
                           (pos * 2 + i) * 256 + (s + 1) * CW],
                        lhsT=x_sb[s][dc][:, nb * 128:(nb + 1) * 128],
                        rhs=wv_sb[s][:, dc, :],
                        start=(dc == 0),
                        stop=(dc == DC - 1),
                    )
        if pos == 1:
            # ISA free-dim patterns are limited to 3D: one copy per head
            for h in range(HPC):
                eng = nc.vector if (eng_ix + h) % 2 == 0 else nc.scalar
                _copy_on(eng, nc,
                         out=v_all[:, quad * 4:(quad + 1) * 4, h, :, :],
                         in_=ps.rearrange("p (n s h d) -> p n h s d",
                                          n=4, s=2, h=HPC)[:, :, h, :, :])

    # pre-phase: only what unit 0's first iterations need — chunk-0 QK
    # projections and the first four V blocks. Everything else (remaining
    # V quads, chunk-1 QK groups) is interleaved into units 0-1.
    for s in range(2):
        for dst, w_sb in ((qt, wq_sb), (kt, wk_sb)):
            qk_group(0, s, dst, w_sb)
    vproj_pair(0, 0, 0)
    vproj_pair(0, 1, 0)
    # the first score matmuls start ~3.5us after the last projection work
    # (evacuation + semaphore latency); bridge that idle window with junk
    # matmuls so the attention stream opens at the full 2.4 GHz clock
    # instead of inside a HAM re-throttle
    warm2 = spool.tile([128, 1024], f32, tag="s", name="warm2")
    for _ in range(24):
        nc.tensor.matmul(warm2[:, 0:128], lhsT=ones_mat, rhs=ones_mat,
                         start=True, stop=True)

    # ---- attention -------------------------------------------------------
    # merged[s]: [128 = (h0 dh | h1 dh), N] per stream, normalized.
    merged = [singles.tile([128, N], bf, tag=f"merged{s}", name=f"merged{s}") for s in range(2)]

    o_stage = [None]

    def outproj_step(qb_src, idx, stage_engines, dma_eng, pair_pos, pool=None,
                     keep_warm=False):
        # idx 0..15 -> stream s = idx//8, row block rb within the q-block;
        # pair_pos 0/1: two consecutive same-stream row blocks share one DMA
        s, r = divmod(idx, 8)
        rb = qb_src * (QB // 128) + r
        if pool is None:
            ps = spool.tile([128, 1024], f32, tag="s", name="ps")
        else:
            ps = pool.tile([128, 1024], f32, tag="av", name="ps")
        nc.tensor.matmul(
            ps[:, 0:512],
            lhsT=merged[s][:, rb * 128:(rb + 1) * 128],
            rhs=wout_sb,
            start=True,
            stop=True,
        )
        if keep_warm:
            # the tail out-projection is staging-gated (~50% PE duty) which
            # lets HAM re-throttle the PE clock; burn a junk matmul into the
            # unused PSUM half to hold the busy window
            nc.tensor.matmul(
                ps[:, 512:1024],
                lhsT=ones_mat,
                rhs=qt[0][:, 0:512],
                start=True,
                stop=True,
            )
        # DMA cannot read PSUM; stage via SBUF, two row-blocks per DMA.
        if pair_pos == 0:
            o_stage[0] = ostage.tile([128, 2, 512], bf, tag="ost", name="ost")
        _copy_on(stage_engines[pair_pos], nc,
                 out=o_stage[0][:, pair_pos, :], in_=ps[:, 0:512])
        if pair_pos == 1:
            r0 = s * N + (rb - 1) * 128
            dma_eng.dma_start(
                out=out[r0:r0 + 256, :].rearrange("(b p) c -> p b c", p=128),
                in_=o_stage[0],
            )

    units = [(qb, h) for qb in range(NQB) for h in range(HPC)]
    pending = [None]
    avtail = [None]

    def make_epilogue(qb, h, av, acc):
        q0 = qb * QB

        def epilogue():
            bc = spool.tile([128, 1024], f32, tag="s", name="bc")
            for qh in range(2):
                nc.tensor.matmul(bc[:, qh * 512:(qh + 1) * 512], lhsT=ones_mat,
                                 rhs=acc[1][:, qh * 512:(qh + 1) * 512],
                                 start=True, stop=True)
            # unnormalized evac frees the AV PSUM; normalize on the
            # otherwise-idle gpsimd engine (it cannot read PSUM)
            un = unpool.tile([128, 1024], bf, tag="un", name="un")
            nc.vector.tensor_copy(out=un, in_=av)
            rec32 = bcpool.tile([128, 1024], f32, tag="rec32", name="rec32")
            nc.vector.reciprocal_approx_fast(out=rec32, in_=bc)
            bc16 = bcpool.tile([128, 1024], bf, tag="bc16", name="bc16")
            nc.vector.tensor_copy(out=bc16, in_=rec32)
            # split per 512-half so the first merged columns land early for
            # the interleaved out-projection of the previous q-block
            for qh in range(2):
                sl = slice(qh * 512, (qh + 1) * 512)
                for s in range(2):
                    nc.gpsimd.tensor_mul(
                        out=merged[s][h * 64:(h + 1) * 64,
                                      q0 + qh * 512:q0 + (qh + 1) * 512],
                        in0=un[s * 64:(s + 1) * 64, sl],
                        in1=bc16[s * 64:(s + 1) * 64, sl],
                    )

        return epilogue

    def emit_av(av, h, pts, kb):
        for qh in range(2):
            nc.tensor.matmul(
                av[:, qh * 512:(qh + 1) * 512],
                lhsT=v_all[:, kb, h, :, :],
                rhs=pts[kb][:, qh * 512:(qh + 1) * 512],
                start=(kb == 0),
                stop=(kb == NKB - 1),
            )

    def emit_add(acc, pts, kb):
        # denominator accumulation (ping-pong, non-in-place) on DVE
        if kb == 1:
            acc[1] = accpool.tile([128, 1024], bf, tag="acc1", name="acc1")
            nc.vector.tensor_add(out=acc[1], in0=pts[0], in1=pts[1])
        elif kb >= 2:
            if acc[kb % 2] is None:
                acc[kb % 2] = accpool.tile([128, 1024], bf,
                                           tag=f"acc{kb % 2}", name="acc")
            nc.vector.tensor_add(out=acc[kb % 2], in0=acc[1 - kb % 2],
                                 in1=pts[kb])

    for u_idx, (qb, h) in enumerate(units):
        q0 = qb * QB
        av = avpool.tile([128, 1024], f32, tag="av", name="av")
        acc = [None, None]  # ping-pong accumulators
        # previous q-block's out-projection interleave order: the rows that
        # only need the qh0 half of unit1's normalize go to unit2, the qh1
        # rows to unit3 (so no PE wait on the gpsimd normalize)
        # previous q-block's out-projection: spread one matmul per two
        # k-iterations across units 2-3 so the PE never co-paces the exp
        # stream; unit2 gets only rows whose normalize half lands early
        if u_idx == 2:
            op_sched = {kb: i for i, kb in enumerate([5, 7, 9, 11, 13, 15])}
            op_order = [0, 1, 8, 9, 2, 3]
        else:
            op_sched = {kb: i for i, kb in
                        enumerate([1, 2, 3, 4, 5, 7, 9, 11, 13, 15])}
            op_order = [10, 11, 4, 5, 12, 13, 6, 7, 14, 15]
        pts = {}
        for kb in range(NKB):
            s_ps = spool.tile([128, 1024], f32, tag="s", name="s")
            # the exp stream is paced by these; keep them at the head of
            # the PE queue
            with tc.high_priority(offset=1 << 20):
                for qh in range(2):
                    nc.tensor.matmul(
                        s_ps[:, qh * 512:(qh + 1) * 512],
                        lhsT=kt[h][:, kb * 128:(kb + 1) * 128],
                        rhs=qt[h][:, q0 + qh * 512:q0 + (qh + 1) * 512],
                        start=True,
                        stop=True,
                    )
            pt = ptpool.tile([128, 1024], bf, tag="pt", name="pt")
            nc.scalar.activation(out=pt, in_=s_ps, func=EXP, scale=SCALE)
            pts[kb] = pt
            # previous unit's AV/add tail goes here so it never blocks this
            # unit's score matmuls in the PE queue
            if kb == 0 and avtail[0] is not None:
                avtail[0]()
                avtail[0] = None
            if kb == 2 and pending[0] is not None:
                pending[0]()
                pending[0] = None
            if u_idx == 0:
                # fill unit 0's PE slack with the remaining V quads and the
                # chunk-1 KT projection groups (needed from kb 8 on)
                if kb in (0, 1):
                    vproj_pair(1, kb, kb)
                elif kb == 2:
                    qk_group(1, 0, kt, wk_sb)
                elif kb in (3, 4):
                    vproj_pair(2, kb - 3, kb)
                elif kb == 5:
                    qk_group(1, 1, kt, wk_sb)
                elif kb in (6, 7):
                    vproj_pair(3, kb - 6, kb)
            elif u_idx == 1:
                # chunk-1 QT groups are only needed by unit 2
                if kb == 1:
                    qk_group(1, 0, qt, wq_sb)
                elif kb == 3:
                    qk_group(1, 1, qt, wq_sb)
            if qb == 1 and kb in op_sched:
                # previous q-block's out-projection (after this iteration's
                # scores so a normalize wait can never stall the exp stream;
                # stage copies on DVE, DMAs on sync)
                i = op_sched[kb]
                outproj_step(0, op_order[i], (nc.vector, nc.vector),
                             nc.sync, i % 2)
            if kb >= 1:
                emit_av(av, h, pts, kb - 1)
                emit_add(acc, pts, kb - 1)
        avtail[0] = (lambda av=av, h=h, acc=acc, pts=pts:
                     (emit_av(av, h, pts, NKB - 1), emit_add(acc, pts, NKB - 1)))
        if u_idx < len(units) - 1:
            pending[0] = make_epilogue(qb, h, av, acc)
        else:
            last_av, last_acc = av, acc

    # ---- tail: last unit's epilogue fused with its out-projection --------
    avtail[0]()
    avtail[0] = None
    qb, h = units[-1]
    q0 = qb * QB
    bc = spool.tile([128, 1024], f32, tag="s", name="bc")
    bcast = bcpool.tile([128, 1024], f32, tag="rec32", name="bcast")
    for qh in range(2):
        nc.tensor.matmul(bc[:, qh * 512:(qh + 1) * 512], lhsT=ones_mat,
                         rhs=last_acc[1][:, qh * 512:(qh + 1) * 512],
                         start=True, stop=True)
    tail_order = [(0, 1), (8, 9), (2, 3), (10, 11), (4, 5), (12, 13), (6, 7), (14, 15)]
    tail_dma = [nc.sync, nc.scalar]
    for qh in range(2):
        sl = slice(qh * 512, (qh + 1) * 512)
        nc.vector.reciprocal_approx_fast(out=bcast[:, sl], in_=bc[:, sl])
        for s in range(2):
            nc.vector.tensor_mul(
                out=merged[s][h * 64:(h + 1) * 64,
                              q0 + qh * 512:q0 + (qh + 1) * 512],
                in0=last_av[s * 64:(s + 1) * 64, sl],
                in1=bcast[s * 64:(s + 1) * 64, sl],
            )
        # row blocks covered by this half can project immediately; psums
        # alternate between the scores ring and the now-free AV banks so
        # the matmuls are not gated on the staging copies
        for n_pair, (i0, i1) in enumerate(tail_order[qh * 4:(qh + 1) * 4]):
            # AV banks are only safe to reuse once every read of the last
            # AV tile has been emitted (i.e. in the qh==1 wave)
            pool = avpool if (qh == 1 and n_pair % 2 == 1) else None
            outproj_step(NQB - 1, i0, (nc.scalar, nc.vector),
                         tail_dma[n_pair % 2], 0, pool=pool, keep_warm=True)
            outproj_step(NQB - 1, i1, (nc.scalar, nc.vector),
                         tail_dma[n_pair % 2], 1, pool=pool, keep_warm=True)

    for p in reversed(pools):
        p.release()


_NC_CACHE = None


def _build():
    global _NC_CACHE
    if _NC_CACHE is None:
        nc = bass.Bass("TRN2", target_bir_lowering=False, debug=False)
        with tile.TileContext(nc) as tc:
            _body(nc, tc)
        # populate .instr bytes for extended InstISA subclasses (the custom
        # DVE reciprocal) — raw bass skips this pass
        mybir.codegen_inst_isa_subclasses(nc)
        _split_sync_waits(nc)
        _NC_CACHE = nc
    return _NC_CACHE


def _prep_in_maps(x1, x2, W_qkv1, W_qkv2, W_out):
    x1 = np.asarray(x1, np.float32)
    x2 = np.asarray(x2, np.float32)
    W1 = np.asarray(W_qkv1, np.float32).astype(BF16)
    W2 = np.asarray(W_qkv2, np.float32).astype(BF16)
    Wo = np.asarray(W_out, np.float32).astype(BF16)
    xT = [
        [np.ascontiguousarray(x[b].T).astype(BF16) for b in range(B)]
        for x in (x1, x2)
    ]
    in_maps = []
    for c in range(NCORES):
        b, hg = divmod(c, NCORES // B)
        cs = slice(hg * CW, (hg + 1) * CW)
        in_maps.append({
            "x1T": xT[0][b],
            "x2T": xT[1][b],
            "wq1": np.ascontiguousarray(W1[:, 0:D][:, cs]),
            "wq2": np.ascontiguousarray(W2[:, 0:D][:, cs]),
            "wk1": np.ascontiguousarray(W1[:, D:2 * D][:, cs]),
            "wk2": np.ascontiguousarray(W2[:, D:2 * D][:, cs]),
            "wv1": np.ascontiguousarray(W1[:, 2 * D:3 * D][:, cs]),
            "wv2": np.ascontiguousarray(W2[:, 2 * D:3 * D][:, cs]),
            "wout": np.ascontiguousarray(Wo[cs, :]),
        })
    return in_maps


def _run(inputs, **spmd_kwargs):
    nc = _build()
    in_maps = _prep_in_maps(
        inputs["x1"], inputs["x2"], inputs["W_qkv1"], inputs["W_qkv2"],
        inputs["W_out"],
    )
    res = run_bass_kernel_spmd(nc, in_maps, core_ids=list(range(NCORES)),
                               **spmd_kwargs)
    b_out = np.asarray(inputs["b_out"], np.float32)
    gpc = NCORES // B
    full = np.zeros((B, 2 * N, D), np.float32)
    for c in range(NCORES):
        full[c // gpc] += res.results[c]["out"].astype(np.float32)
    full += b_out
    return full, res


def kernel(**inputs):
    full, _ = _run(inputs)
    return full
